# revision 14
# baseline (speedup 1.0000x reference)
"""Trainium2 Bass kernel for the DGCNN subject/predicate extraction model.

Strategy: data-parallel over batch (16 batch elems -> 8 cores x 2).
On-device layout is "CT": channels on SBUF partitions, sequence on the free
dim, so every conv tap is a plain matmul against a shifted slice of a
zero-padded sequence buffer and no transposes are needed between layers.
Matmuls run in float32r (full PE rate at N>=256, near-fp32 accuracy).

Attention: heads padded to 32 rows for Q/K (so per-head slices are
32-partition aligned) and 64 rows for V/O.  The V bias row carries a 1.0 in
pad column h*64+32, which makes the PV matmul accumulate the softmax
denominator into o[h*64+32, :] for free; normalization is then a
per-64-row-block multiply by a PE-broadcast reciprocal row.
"""

import numpy as np

import concourse.bass as bass
import concourse.mybir as mybir
import concourse.tile as tile
from concourse.vector_clock import ScopedClock
from concourse.bass_utils import run_bass_kernel_spmd

# ---------------------------------------------------------------------------
# Workaround: this walrus build rejects instructions carrying more than one
# semaphore wait command (setupSyncWait "Too many sync wait commands") and the
# TileContext exit drain aggregates one wait per DMA lane.  Split the extra
# waits onto dedicated nops on the same engine (order of blocking conditions
# on one engine does not change the final state).
_ORIG_DRAIN = tile.TileContext._drain_and_barrier


def _patched_drain_and_barrier(self, tick_clock, wait_clock):
    nc = self.nc
    drain_inst = nc.sync.drain()
    wait_clock.add_sem_waits(
        drain_inst.ins, ScopedClock({None: tick_clock.global_clock})
    )
    si = drain_inst.ins.sync_info
    waits = list(si.on_wait or [])
    if len(waits) > 1:
        si.on_wait = [waits[0]]
        for w in waits[1:]:
            nop = nc.sync.nop(nofuse=True)
            nsi = nop.ins.sync_info
            if nsi is None:
                nop.ins.sync_info = mybir.SyncInfo(on_wait=[w], on_update=[])
            else:
                nsi.on_wait = [w]
    nc.all_engine_barrier()
    popped = nc._tile_sem_poison_stack.pop()
    assert popped is self._sem_poison
    nc.clear_and_free_semaphores(list(self.sems.allocated().values()))
    nc.all_engine_barrier()


tile.TileContext._drain_and_barrier = _patched_drain_and_barrier


def _split_multi_waits(nc):
    """This walrus build supports a single sync-wait command per instruction;
    hoist extra waits onto dedicated EventSemaphore instructions just before
    the owning instruction on the same engine (an engine blocks on each wait
    in sequence, so the final state is identical)."""
    n = 0
    for fn in nc.m.functions:
        for bb in fn.blocks:
            out = []
            for ins in bb.instructions:
                si = ins.sync_info
                waits = list(si.on_wait or []) if si is not None else []
                if len(waits) > 1:
                    for w in waits[:-1]:
                        es = mybir.InstEventSemaphore(
                            name=f"wsplit_{n}", ins=[], outs=[]
                        )
                        n += 1
                        es.engine = ins.engine
                        es.sync_info = mybir.SyncInfo(on_wait=[w], on_update=[])
                        out.append(es)
                    si.on_wait = [waits[-1]]
                out.append(ins)
            bb.instructions = out

# ---------------------------------------------------------------------------

F32 = mybir.dt.float32
I32 = mybir.dt.int32
R = mybir.dt.float32r

P = 128
S = 512
PAD = 8
SP = S + 2 * PAD  # 528
NB = 2  # batch elems per core
NCORES = 8
B = 16
DILATIONS = (1, 2, 5, 1, 2, 5, 1, 2, 5, 1, 1, 1)
NL = 12
DG = 256  # dgc channels
H = 8  # heads (both attentions)
K1 = 16  # att1 key dim
K2 = 24  # att2 key dim
HQ = H * 32  # 256: q/k padded width
HV = H * 64  # 512: v/o padded width

Sig = mybir.ActivationFunctionType.Sigmoid
Exp = mybir.ActivationFunctionType.Exp
Relu = mybir.ActivationFunctionType.Relu
Ident = mybir.ActivationFunctionType.Identity


def _r(ap):
    return ap.bitcast(R)


def _build(masking: bool):
    from concourse.masks import make_identity

    nc = bass.Bass(target_bir_lowering=False)

    def din(name, shape, dt=F32):
        return nc.dram_tensor(name, shape, dt, kind="ExternalInput")

    ce = din("ce", [8000, 128])
    we = din("we", [50000, 128])
    pe = din("pe", [512, 256])
    idxc = din("idxc", [NB, 4, P], I32)
    idxw = din("idxw", [NB, 4, P], I32)
    idxp = din("idxp", [NB, 4, P], I32)
    dgcw = din("dgcw", [NL, DG, 3 * 2 * DG])
    dgcb = din("dgcb", [NL, P, 4])
    a1wq = din("a1wq", [DG, HQ])
    a1wk = din("a1wk", [DG, HQ])
    a1wv = din("a1wv", [DG, HV])
    a1wo = din("a1wo", [HV, DG])
    a1bq = din("a1bq", [P, 2])
    a1bk = din("a1bk", [P, 2])
    a1bv = din("a1bv", [1, HV])
    a1bo = din("a1bo", [P, 2])
    D2 = 384
    a2wq = din("a2wq", [D2, HQ])
    a2wk = din("a2wk", [D2, HQ])
    a2wv = din("a2wv", [D2, HV])
    a2wo = din("a2wo", [HV, D2])
    a2bq = din("a2bq", [P, 2])
    a2bk = din("a2bk", [P, 2])
    a2bv = din("a2bv", [1, HV])
    a2bo = din("a2bo", [P, 3])
    c1w = din("c1w", [512, 3 * 128])
    c1b = din("c1b", [P, 1])
    subw = din("subw", [P, 2])
    subb = din("subb", [2, 1])
    c2w = din("c2w", [768, 3 * 256])
    c2b = din("c2b", [P, 2])
    pow_ = din("pow", [DG, 100])
    pob = din("pob", [100, 1])
    oneh = din("oneh", [NB, 4, P, 2])
    if masking:
        maskr = din("maskr", [NB, 1, S])
        penc = din("penc", [NB, P, 4])
    osub = nc.dram_tensor("osub", [NB, 2, S], F32, kind="ExternalOutput")
    opo = nc.dram_tensor("opo", [NB, 100, S], F32, kind="ExternalOutput")

    with tile.TileContext(nc) as tc:
        with (
            nc.allow_low_precision(reason="float32r matmul inputs (same bits as f32)"),
            tc.tile_pool(name="wres", bufs=1) as wres,
            tc.tile_pool(name="wdgc", bufs=4) as wdgc,
            tc.tile_pool(name="acts", bufs=1) as acts,
            tc.tile_pool(name="tmp", bufs=2) as tmp,
            tc.tile_pool(name="tmp1", bufs=1) as tmp1,
            tc.tile_pool(name="psg", bufs=2, space="PSUM") as psg,
            tc.tile_pool(name="pslg", bufs=2, space="PSUM") as pslg,
            tc.tile_pool(name="pso", bufs=1, space="PSUM") as pso,
        ):
            # ---- constants ----
            ident = wres.tile([P, P], F32, tag="ident")
            make_identity(nc, ident[:])
            onesf = wres.tile([1, P], F32, tag="onesf")
            nc.vector.memset(onesf[:], 1.0)
            ones1 = wres.tile([1, P], R, tag="ones1")
            nc.vector.tensor_copy(ones1[:], onesf[:])
            identr = wres.tile([P, P], R, tag="identr")
            nc.vector.tensor_copy(identr[:], ident[:])
            zpadf = wres.tile([P, PAD], F32, tag="zpadf")
            nc.vector.memset(zpadf[:], 0.0)

            # ---- resident weights ----
            def load_w(dram, rows, cols, tag):
                nt = (rows + P - 1) // P
                ts = []
                for kt in range(nt):
                    t = wres.tile([P, cols], R, tag=f"{tag}{kt}")
                    nc.sync.dma_start(
                        out=t[:], in_=dram[kt * P : (kt + 1) * P, :].bitcast(R)
                    )
                    ts.append(t)
                return ts

            def load_small(dram, shape, tag, dt=F32):
                t = wres.tile(list(shape), dt, tag=tag)
                nc.sync.dma_start(out=t[:], in_=dram[:].bitcast(dt))
                return t

            w1q = load_w(a1wq, DG, HQ, "w1q")
            w1k = load_w(a1wk, DG, HQ, "w1k")
            w1v = load_w(a1wv, DG, HV, "w1v")
            w1o = load_w(a1wo, HV, DG, "w1o")
            w2q = load_w(a2wq, D2, HQ, "w2q")
            w2k = load_w(a2wk, D2, HQ, "w2k")
            w2v = load_w(a2wv, D2, HV, "w2v")
            w2o = load_w(a2wo, HV, D2, "w2o")
            wc1 = load_w(c1w, 512, 3 * 128, "wc1")
            wpo = load_w(pow_, DG, 100, "wpo")
            b1q = load_small(a1bq, (P, 2), "b1q")
            b1k = load_small(a1bk, (P, 2), "b1k")
            b1v = load_small(a1bv, (1, HV), "b1v", R)
            b1o = load_small(a1bo, (P, 2), "b1o")
            b2q = load_small(a2bq, (P, 2), "b2q")
            b2k = load_small(a2bk, (P, 2), "b2k")
            b2v = load_small(a2bv, (1, HV), "b2v", R)
            b2o = load_small(a2bo, (P, 3), "b2o")
            bc1 = load_small(c1b, (P, 1), "bc1")
            wsub = load_small(subw, (P, 2), "wsub", R)
            bsub = load_small(subb, (2, 1), "bsub")
            bc2 = load_small(c2b, (P, 2), "bc2")
            bpo = load_small(pob, (100, 1), "bpo")
            ohsb = []
            for b in range(NB):
                t = wres.tile([P, 8], R, tag=f"oh{b}")
                for st in range(4):
                    nc.sync.dma_start(
                        out=t[:, st * 2 : st * 2 + 2], in_=oneh[b, st].bitcast(R)
                    )
                ohsb.append(t)
            if masking:
                pensb = []
                for b in range(NB):
                    t = wres.tile([P, 4], F32, tag=f"pen{b}")
                    nc.sync.dma_start(out=t[:], in_=penc[b])
                    pensb.append(t)

            # ---- persistent activation buffers (padded CT seq buffers) ----
            def padded_tile(tag):
                t = acts.tile([P, SP], R, tag=tag)
                nc.vector.tensor_copy(t[:, 0:PAD], zpadf[:])
                nc.vector.tensor_copy(t[:, PAD + S : SP], zpadf[:])
                return t

            # x ping/pong: x[b][pp][ci]
            xts = [
                [[padded_tile(f"x{b}p{pp}c{ci}") for ci in range(2)] for pp in range(2)]
                for b in range(NB)
            ]
            def big_tile(name):
                t = acts.tile([P, SP], R, tag="big528", name=name, bufs=8)
                nc.vector.tensor_copy(t[:, 0:PAD], zpadf[:])
                nc.vector.tensor_copy(t[:, PAD + S : SP], zpadf[:])
                return t

            maskb = []
            if masking:
                for b in range(NB):
                    mrow = tmp1.tile([1, S], F32, tag=f"mrow{b}")
                    nc.sync.dma_start(out=mrow[:], in_=maskr[b])
                    ps = psg.tile([P, S], F32, tag="gp")
                    nc.tensor.matmul(
                        ps[:], _r(ones1[:1, :]), _r(mrow[:1, :]), start=True, stop=True
                    )
                    mb = acts.tile([P, S], R, tag=f"maskb{b}")
                    nc.vector.tensor_copy(mb[:], ps[:])
                    maskb.append(mb)

            # ---- embeddings ----
            for b in range(NB):
                # gather [s-tile, emb] tiles then transpose-accumulate into CT
                cg, wg, pg = [], [], []
                for st in range(4):
                    ic = tmp.tile([P, 1], I32, tag="ixc")
                    iw = tmp.tile([P, 1], I32, tag="ixw")
                    ip = tmp.tile([P, 1], I32, tag="ixp")
                    nc.sync.dma_start(out=ic[:], in_=idxc[b, st][:, None])
                    nc.sync.dma_start(out=iw[:], in_=idxw[b, st][:, None])
                    nc.sync.dma_start(out=ip[:], in_=idxp[b, st][:, None])
                    cgt = tmp.tile([P, 128], F32, tag=f"cg{st}", bufs=1)
                    wgt = tmp.tile([P, 128], F32, tag=f"wg{st}", bufs=1)
                    pgt = tmp.tile([P, 256], F32, tag=f"pg{st}", bufs=1)
                    nc.gpsimd.indirect_dma_start(
                        out=cgt[:], out_offset=None, in_=ce[:, :],
                        in_offset=bass.IndirectOffsetOnAxis(ap=ic[:, :1], axis=0),
                    )
                    nc.gpsimd.indirect_dma_start(
                        out=wgt[:], out_offset=None, in_=we[:, :],
                        in_offset=bass.IndirectOffsetOnAxis(ap=iw[:, :1], axis=0),
                    )
                    nc.gpsimd.indirect_dma_start(
                        out=pgt[:], out_offset=None, in_=pe[:, :],
                        in_offset=bass.IndirectOffsetOnAxis(ap=ip[:, :1], axis=0),
                    )
                    cg.append(cgt)
                    wg.append(wgt)
                    pg.append(pgt)
                for ct in range(2):
                    ps = psg.tile([P, S], F32, tag="gp")
                    for st in range(4):
                        main = cg[st][:, :] if ct == 0 else wg[st][:, :]
                        sl = slice(st * P, (st + 1) * P)
                        nc.tensor.matmul(
                            ps[:, sl], main, ident[:], is_transpose=True,
                            start=True, stop=False,
                        )
                        nc.tensor.matmul(
                            ps[:, sl], pg[st][:, ct * 128 : (ct + 1) * 128],
                            ident[:], is_transpose=True, start=False, stop=True,
                        )
                    dst = xts[b][0][ct][:, PAD : PAD + S]
                    if masking:
                        nc.vector.tensor_mul(dst, ps[:], maskb[b][:])
                    else:
                        nc.vector.tensor_copy(dst, ps[:])

            # ---- DGC stack ----
            for li, d in enumerate(DILATIONS):
                wt = []
                for ci in range(2):
                    t = wdgc.tile([P, 3 * 2 * DG], R, tag=f"wdgc{ci}")
                    nc.sync.dma_start(
                        out=t[:], in_=dgcw[li, ci * P : (ci + 1) * P, :].bitcast(R)
                    )
                    wt.append(t)
                bcol = wdgc.tile([P, 4], F32, tag="dgcb")
                nc.sync.dma_start(out=bcol[:], in_=dgcb[li])
                for b in range(NB):
                    xin = xts[b][li % 2]
                    xout = xts[b][(li + 1) % 2]
                    gsb = tmp.tile([P, 2 * S], F32, tag="gsb", bufs=1)
                    for cot in range(4):
                        ps = psg.tile([P, S], F32, tag="gp")
                        k = 0
                        for w in range(3):
                            off = PAD + (w - 1) * d
                            for ci in range(2):
                                nc.tensor.matmul(
                                    ps[:],
                                    _r(wt[ci][:, w * 2 * DG + cot * P : w * 2 * DG + (cot + 1) * P]),
                                    _r(xin[ci][:, off : off + S]),
                                    start=(k == 0), stop=(k == 5),
                                )
                                k += 1
                        if cot < 2:
                            nc.scalar.activation(
                                out=gsb[:, cot * S : (cot + 1) * S], in_=ps[:],
                                func=Sig, bias=bcol[:, cot : cot + 1], scale=1.0,
                            )
                        else:
                            co = cot - 2
                            xm = xin[co][:, PAD : PAD + S]
                            t1 = tmp.tile([P, S], F32, tag="t1")
                            nc.vector.tensor_scalar_add(
                                t1[:], ps[:], bcol[:, cot : cot + 1]
                            )
                            nc.vector.tensor_sub(t1[:], t1[:], xm)
                            nc.vector.tensor_mul(
                                t1[:], t1[:], gsb[:, co * S : (co + 1) * S]
                            )
                            dst = xout[co][:, PAD : PAD + S]
                            if masking:
                                nc.vector.tensor_add(t1[:], t1[:], xm)
                                nc.vector.tensor_mul(dst, t1[:], maskb[b][:])
                            else:
                                nc.vector.tensor_add(dst, t1[:], xm)

            # ---- attention (shared) ----
            def attention(b, x_tiles, wq, wk, wv, wo, bqc, bkc, bvr, boc,
                          key_dim, out_writer):
                n_ci = len(x_tiles)
                scale = 1.0 / float(np.sqrt(key_dim))
                qh, kh = [], []
                for which, (wmat, bcolt, outl) in enumerate(
                    ((wq, bqc, qh), (wk, bkc, kh))
                ):
                    for mt in range(2):
                        ps = psg.tile([P, S], F32, tag="gp")
                        for ci in range(n_ci):
                            nc.tensor.matmul(
                                ps[:],
                                _r(wmat[ci][:, mt * P : (mt + 1) * P]),
                                _r(x_tiles[ci][:, PAD : PAD + S]),
                                start=(ci == 0), stop=(ci == n_ci - 1),
                            )
                        t = tmp1.tile([P, S], R, tag=f"qk{which}{mt}")
                        nc.scalar.activation(
                            out=t[:], in_=ps[:], func=Ident,
                            bias=bcolt[:, mt : mt + 1], scale=1.0,
                        )
                        outl.append(t)
                vh = []
                for st in range(4):
                    ps = psg.tile([P, HV], F32, tag="gp")
                    for ci in range(n_ci):
                        nc.tensor.matmul(
                            ps[:],
                            _r(x_tiles[ci][:, PAD + st * P : PAD + (st + 1) * P]),
                            _r(wv[ci][:]),
                            start=(ci == 0), stop=False,
                        )
                    nc.tensor.matmul(
                        ps[:], _r(ones1[:1, :]), _r(bvr[:1, :]),
                        start=False, stop=True,
                    )
                    t = tmp1.tile([P, HV], R, tag=f"vh{st}")
                    nc.vector.tensor_copy(t[:], ps[:])
                    vh.append(t)
                osb = [
                    tmp1.tile([P, S], R, tag=f"osb{i}", name=f"osb{i}")
                    for i in range(4)
                ]

                def logits_exp(h):
                    mt, lo = divmod(h, 4)
                    lo *= 32
                    et = tmp.tile([P, 4 * S], R, tag="expt")
                    for st in range(4):
                        lp = pslg.tile([P, S], F32, tag="lg")
                        nc.tensor.matmul(
                            lp[:],
                            _r(kh[mt][lo : lo + 32, st * P : (st + 1) * P]),
                            _r(qh[mt][lo : lo + 32, :]),
                            start=True, stop=True,
                            tile_position=(lo, 0),
                        )
                        bias = (
                            pensb[b][:, st : st + 1] if masking else 0.0
                        )
                        nc.scalar.activation(
                            out=et[:, st * S : (st + 1) * S], in_=lp[:],
                            func=Exp, bias=bias, scale=scale,
                        )
                    return et

                def pv_norm(h, et):
                    oti = pso.tile([64, S], F32, tag="ot", name=f"ot{h}", bufs=2)
                    lo = (h % 2) * 64
                    for st in range(4):
                        nc.tensor.matmul(
                            oti[:],
                            _r(vh[st][:, h * 64 : (h + 1) * 64]),
                            _r(et[:, st * S : (st + 1) * S]),
                            start=(st == 0), stop=(st == 3),
                        )
                    rc = tmp.tile([1, S], R, tag="rc")
                    nc.vector.reciprocal(rc[:], oti[32:33, :])
                    bc = psg.tile([P, S], F32, tag="gp")
                    nc.tensor.matmul(
                        bc[0:64, :], _r(ones1[:1, 0:64]), _r(rc[:1, :]),
                        start=True, stop=True,
                    )
                    bcs = tmp.tile([64, S], R, tag="bcs")
                    nc.scalar.copy(bcs[:], bc[0:64, :])
                    nc.vector.tensor_mul(
                        osb[h // 2][lo : lo + 64, :], oti[:], bcs[:],
                    )

                # software-pipelined head loop
                et_prev = logits_exp(0)
                for h in range(H):
                    et_next = logits_exp(h + 1) if h + 1 < H else None
                    pv_norm(h, et_prev)
                    et_prev = et_next
                # out-projection
                n_mt = wo[0].shape[1] // P
                for mt in range(n_mt):
                    ps = psg.tile([P, S], F32, tag="gp")
                    for kt in range(4):
                        nc.tensor.matmul(
                            ps[:],
                            _r(wo[kt][:, mt * P : (mt + 1) * P]),
                            _r(osb[kt][:]),
                            start=(kt == 0), stop=(kt == 3),
                        )
                    out_writer(mt, ps)

            wc2 = []
            for i in range(6):
                t = wdgc.tile([P, 3 * 256], R, tag=f"wdgc{i % 2}",
                              name=f"wc2_{i}")
                nc.sync.dma_start(out=t[:], in_=c2w[i * P : (i + 1) * P, :].bitcast(R))
                wc2.append(t)

            for b in range(NB):
                xf = xts[b][0]  # final dgc output (12 layers -> pp 0)
                attt_b = [big_tile(f"attt{b}m{mt}") for mt in range(2)]

                def att1_writer(mt, ps, b=b, attt_b=attt_b):
                    nc.scalar.activation(
                        out=attt_b[mt][:, PAD : PAD + S], in_=ps[:],
                        func=Ident, bias=b1o[:, mt : mt + 1], scale=1.0,
                    )

                attention(b, xf, w1q, w1k, w1v, w1o, b1q, b1k, b1v, b1o,
                          K1, att1_writer)

                # conv1: channels = [x (256), att (256)]
                poft_b = [big_tile(f"poft{b}m{mt}") for mt in range(3)]
                cin1 = [xf[0], xf[1], attt_b[0], attt_b[1]]
                ps = psg.tile([P, S], F32, tag="gp")
                k = 0
                for w in range(3):
                    off = PAD + (w - 1)
                    for ci in range(4):
                        nc.tensor.matmul(
                            ps[:],
                            _r(wc1[ci][:, w * 128 : (w + 1) * 128]),
                            _r(cin1[ci][:, off : off + S]),
                            start=(k == 0), stop=(k == 11),
                        )
                        k += 1
                nc.scalar.activation(
                    out=poft_b[0][:, PAD : PAD + S], in_=ps[:], func=Relu,
                    bias=bc1[:, 0:1], scale=1.0,
                )

                # sub_preds head
                ps2 = psg.tile([P, S], F32, tag="gp")
                nc.tensor.matmul(
                    ps2[0:2, :], _r(wsub[:]), _r(poft_b[0][:, PAD : PAD + S]),
                    start=True, stop=True,
                )
                ssb = tmp.tile([2, S], F32, tag="ssb", bufs=1)
                nc.scalar.activation(
                    out=ssb[:], in_=ps2[0:2, :], func=Sig, bias=bsub[:, 0:1],
                    scale=1.0,
                )
                nc.sync.dma_start(out=osub[b], in_=ssb[:])

                # att_dgc transposed (TC) for the sub_start/end gather
                pst = psg.tile([P, S], F32, tag="gp")
                for st in range(4):
                    nc.tensor.matmul(
                        pst[:, st * P : (st + 1) * P].bitcast(R),
                        poft_b[0][:, PAD + st * P : PAD + (st + 1) * P],
                        identr[:], is_transpose=True, start=True, stop=True,
                    )
                adt = tmp.tile([P, S], R, tag="adt", bufs=1)
                nc.vector.tensor_copy(adt[:], pst[:])
                pse = psg.tile([P, S], F32, tag="gp")
                for st in range(4):
                    nc.tensor.matmul(
                        pse[:, 0:2],
                        _r(adt[:, st * P : (st + 1) * P]),
                        _r(ohsb[b][:, st * 2 : st * 2 + 2]),
                        start=(st == 0), stop=(st == 3),
                    )
                sse = tmp.tile([P, 2], F32, tag="sse")
                nc.vector.tensor_copy(sse[:], pse[:, 0:2])
                # broadcast sse columns across the sequence
                for cidx in range(2):
                    nc.scalar.activation(
                        out=poft_b[1 + cidx][:, PAD : PAD + S],
                        in_=poft_b[0][:, PAD : PAD + S], func=Ident,
                        bias=sse[:, cidx : cidx + 1], scale=0.0,
                    )

                pof2_b = [big_tile(f"pof2{b}m{mt}") for mt in range(3)]

                def att2_writer(mt, ps, b=b, pof2_b=pof2_b):
                    nc.scalar.activation(
                        out=pof2_b[mt][:, PAD : PAD + S], in_=ps[:],
                        func=Ident, bias=b2o[:, mt : mt + 1], scale=1.0,
                    )

                attention(b, poft_b, w2q, w2k, w2v, w2o, b2q, b2k, b2v, b2o,
                          K2, att2_writer)

                # conv2: channels = [po_att (384), po_feat (384)]
                cin2 = pof2_b + poft_b
                po2 = []
                for cot in range(2):
                    psc = psg.tile([P, S], F32, tag="gp")
                    k = 0
                    for w in range(3):
                        off = PAD + (w - 1)
                        for ci in range(6):
                            nc.tensor.matmul(
                                psc[:],
                                _r(wc2[ci][:, w * 256 + cot * P : w * 256 + (cot + 1) * P]),
                                _r(cin2[ci][:, off : off + S]),
                                start=(k == 0), stop=(k == 17),
                            )
                            k += 1
                    t = tmp.tile([P, S], R, tag=f"po2{cot}", bufs=1)
                    nc.scalar.activation(
                        out=t[:], in_=psc[:], func=Relu,
                        bias=bc2[:, cot : cot + 1], scale=1.0,
                    )
                    po2.append(t)
                psp = psg.tile([P, S], F32, tag="gp")
                for kt in range(2):
                    nc.tensor.matmul(
                        psp[0:100, :], _r(wpo[kt][:]), _r(po2[kt][:]),
                        start=(kt == 0), stop=(kt == 1),
                    )
                pot = tmp.tile([100, S], F32, tag="pot", bufs=1)
                nc.scalar.activation(
                    out=pot[:], in_=psp[0:100, :], func=Sig, bias=bpo[:, 0:1],
                    scale=1.0,
                )
                nc.sync.dma_start(out=opo[b], in_=pot[:])

    _split_multi_waits(nc)
    return nc


_BUILD_CACHE = {}


def _get_nc(masking: bool):
    if masking not in _BUILD_CACHE:
        _BUILD_CACHE[masking] = _build(masking)
    return _BUILD_CACHE[masking]


def _pad_qk_w(w, k):
    # [d, h, k] -> [d, h*32]
    d = w.shape[0]
    out = np.zeros((d, H * 32), np.float32)
    for i in range(H):
        out[:, i * 32 : i * 32 + k] = w[:, i, :]
    return out


def _pad_v_w(w, k):
    d = w.shape[0]
    out = np.zeros((d, H * 64), np.float32)
    for i in range(H):
        out[:, i * 64 : i * 64 + k] = w[:, i, :]
    return out


def _pad_o_w(w, k):
    # [h, k, dout] -> [h*64, dout]
    dout = w.shape[2]
    out = np.zeros((H * 64, dout), np.float32)
    for i in range(H):
        out[i * 64 : i * 64 + k, :] = w[i]
    return out


def _pad_qk_b(bv, k):
    out = np.zeros((H * 32,), np.float32)
    for i in range(H):
        out[i * 32 : i * 32 + k] = bv[i]
    return _cols(out)


def _pad_v_b(bv, k):
    out = np.zeros((1, H * 64), np.float32)
    for i in range(H):
        out[0, i * 64 : i * 64 + k] = bv[i]
        out[0, i * 64 + 32] = 1.0  # softmax-denominator ones column
    return out


def _cols(v):
    # [C] -> [128, C//128] column-tile layout
    v = np.asarray(v, np.float32)
    nt = v.shape[0] // P
    return np.ascontiguousarray(v.reshape(nt, P).T)


def _f(a):
    return np.ascontiguousarray(np.asarray(a), dtype=np.float32)


def kernel(params, inputs, inputs_word, inputs_position, input_sub_loc):
    p = params
    inputs = np.asarray(inputs)
    inputs_word = np.asarray(inputs_word)
    inputs_position = np.asarray(inputs_position)
    input_sub_loc = np.asarray(input_sub_loc)

    mask = inputs != 0
    masking = not bool(mask.all())

    # ---- shared (replicated) weight prep ----
    dgc_w = np.stack([_f(w) for w in p["dgc_k"]])  # [12, 3, 256, 512]
    dgc_w = np.ascontiguousarray(
        dgc_w.transpose(0, 2, 1, 3).reshape(NL, DG, 3 * 2 * DG)
    )
    dgc_b = np.stack([_f(bv) for bv in p["dgc_b"]])  # [12, 512]
    dgc_b = np.ascontiguousarray(dgc_b.reshape(NL, 4, P).transpose(0, 2, 1))

    a1, a2 = p["att1"], p["att2"]
    shared = {
        "ce": _f(p["char_emb"]),
        "we": _f(p["word_emb"]),
        "pe": _f(p["pos_emb"]),
        "dgcw": dgc_w,
        "dgcb": dgc_b,
        "a1wq": _pad_qk_w(_f(a1["Wq"]), K1),
        "a1wk": _pad_qk_w(_f(a1["Wk"]), K1),
        "a1wv": _pad_v_w(_f(a1["Wv"]), K1),
        "a1wo": _pad_o_w(_f(a1["Wo"]), K1),
        "a1bq": _pad_qk_b(_f(a1["bq"]), K1),
        "a1bk": _pad_qk_b(_f(a1["bk"]), K1),
        "a1bv": _pad_v_b(_f(a1["bv"]), K1),
        "a1bo": _cols(_f(a1["bo"])),
        "a2wq": _pad_qk_w(_f(a2["Wq"]), K2),
        "a2wk": _pad_qk_w(_f(a2["Wk"]), K2),
        "a2wv": _pad_v_w(_f(a2["Wv"]), K2),
        "a2wo": _pad_o_w(_f(a2["Wo"]), K2),
        "a2bq": _pad_qk_b(_f(a2["bq"]), K2),
        "a2bk": _pad_qk_b(_f(a2["bk"]), K2),
        "a2bv": _pad_v_b(_f(a2["bv"]), K2),
        "a2bo": _cols(_f(a2["bo"])),
        "c1w": np.ascontiguousarray(
            _f(p["conv1_k"]).transpose(1, 0, 2).reshape(512, 3 * 128)
        ),
        "c1b": _cols(_f(p["conv1_b"])),
        "subw": _f(p["sub_w"]),
        "subb": _f(p["sub_b"]).reshape(2, 1),
        "c2w": np.ascontiguousarray(
            _f(p["conv2_k"]).transpose(1, 0, 2).reshape(768, 3 * 256)
        ),
        "c2b": _cols(_f(p["conv2_b"])),
        "pow": _f(p["po_w"]),
        "pob": _f(p["po_b"]).reshape(100, 1),
    }

    idxc = np.ascontiguousarray(inputs.astype(np.int32).reshape(B, 4, P))
    idxw = np.ascontiguousarray(inputs_word.astype(np.int32).reshape(B, 4, P))
    idxp = np.ascontiguousarray(
        inputs_position.astype(np.int32).reshape(B, 4, P)
    )
    oneh = np.zeros((B, S, 2), np.float32)
    bidx = np.arange(B)
    oneh[bidx, input_sub_loc[:, 0].astype(np.int64), 0] = 1.0
    oneh[bidx, input_sub_loc[:, 1].astype(np.int64), 1] = 1.0
    oneh = np.ascontiguousarray(oneh.reshape(B, 4, P, 2))

    in_maps = []
    for c in range(NCORES):
        bs = slice(c * NB, (c + 1) * NB)
        m = dict(shared)
        m["idxc"] = idxc[bs]
        m["idxw"] = idxw[bs]
        m["idxp"] = idxp[bs]
        m["oneh"] = oneh[bs]
        if masking:
            maskf = mask.astype(np.float32)
            pen = np.where(mask, 0.0, -1e9).astype(np.float32)
            m["maskr"] = np.ascontiguousarray(maskf[bs].reshape(NB, 1, S))
            m["penc"] = np.ascontiguousarray(
                pen[bs].reshape(NB, 4, P).transpose(0, 2, 1)
            )
        in_maps.append(m)

    nc = _get_nc(masking)
    res = run_bass_kernel_spmd(nc, in_maps, core_ids=list(range(NCORES)))
    sub = np.concatenate([r["osub"] for r in res.results], axis=0)
    po = np.concatenate([r["opo"] for r in res.results], axis=0)
    return sub, po, mask
